# revision 1
# baseline (speedup 1.0000x reference)
"""Trainium2 Bass kernel for nn_DGM (deformable-conv guided module).

Sharding: 8 cores = 4 samples x 2 height-halves (pure data parallel, no
collectives). Each core computes out[b, :, h0:h0+64, :] of out = x + dcn(x,
offsets(cat(x, inter))) + x*gamma(inter) + beta(inter).

Per-core pipeline:
  A. offset conv (3x3, 128ch -> 27ch) as 9 PSUM-accumulated matmuls over a
     zero-padded channel-major input tile; per 512-pix tile the 27 offset
     planes are immediately PE-transposed to pixel-major om_pm [128pix, 27].
  B. index/weight pipeline on wide [128, 576] (pixel, (row-tile, tap)) tiles:
     robust floor (works for trunc or round-to-nearest casts), per-corner
     validity, bilinear*mask*validity corner weights, clamped flat gather
     indices into a zero-row-prefixed pixel-major copy of the full sample.
  C. dma_gather: per (pixel, tap) two 512B gathers (pixel-pair rows y0,y0+1);
     indices folded to the SWDGE wrapped-16 layout via 8+7 SBUF DMAs.
  D. blend 4 corners with broadcast-AP tensor_tensor chains (pixels on
     partitions), PE-transpose blended samples to channel-major.
  E. DCN matmul (tap-pair stacked K=128) + SFT 1x1 convs (leaky relu via
     max(x, 0.1x)) + fused residual combine; beta2 is accumulated into the
     DCN PSUM. Stream out per 256-pixel chunk.
"""

import sys

for _p in ("/opt/trn_rl_repo",):
    if _p not in sys.path:
        sys.path.insert(0, _p)

import numpy as np

import concourse.bacc as bacc
import concourse.bass as bass
import concourse.mybir as mybir
from concourse.bass_types import AP
from concourse.tile import TileContext

F32 = mybir.dt.float32
I32 = mybir.dt.int32
I16 = mybir.dt.int16

B, C, H, W, K = 4, 64, 128, 128, 3
K2 = K * K  # 9
HHALF = H // 2  # 64 rows per core
NPIX = HHALF * W  # 8192 pixels per core
NCORES = 8

T_CHUNK = 2                      # 128-pix row-tiles per gather chunk
N_CHUNKS = HHALF // T_CHUNK      # 32
NPC = T_CHUNK * 128              # pixels per chunk (256)
NIDX = T_CHUNK * 2 * K2 * 128    # 4608 gather slots per chunk
NGRP = T_CHUNK * 2 * K2          # 36 gathered groups per chunk

XG_ROWS = W + H * W + 2 * W      # zero prefix row + plane + pad
NTK = HHALF * K2                 # 576 = (row-tile, tap) plane width
RW = W + 2                       # padded row pitch of conv-input tile (130)
NROW = HHALF + 2                 # 66 halo rows

_AL = mybir.AluOpType
_AF = mybir.ActivationFunctionType


def fap(t, foff, dims, p0=0, pn=None):
    """View of tile/dram AP `t` with custom free dims [[step, count], ...].

    foff: offset in elements within a partition row; p0/pn: partition
    slice. Uses t's own partition stride.
    """
    L = t.ap[0][0]
    if pn is None:
        pn = t.ap[0][1]
    return AP(t.tensor, p0 * L + foff, [[L, pn]] + [list(d) for d in dims])


def build_nc(phase=99):
    """phase: 1=conv+transpose, 2=+pipeline/idx, 3=+gather only,
    4=+blend/transpose, 99=full."""
    nc = bacc.Bacc("TRN2", target_bir_lowering=False)

    xg = nc.dram_tensor("xg", [XG_ROWS, C], F32, kind="ExternalInput")
    xi = nc.dram_tensor("xi", [C, NROW * W], F32, kind="ExternalInput")
    ii = nc.dram_tensor("ii", [C, NROW * W], F32, kind="ExternalInput")
    cyf = nc.dram_tensor("cyf", [128, NTK], F32, kind="ExternalInput")
    cxf = nc.dram_tensor("cxf", [128, NTK], F32, kind="ExternalInput")
    wo = nc.dram_tensor("wo", [2 * C, K2 * 27], F32, kind="ExternalInput")
    bo = nc.dram_tensor("bo", [27, 1], F32, kind="ExternalInput")
    wd = nc.dram_tensor("wd", [2 * C, 5 * C], F32, kind="ExternalInput")
    ws1 = nc.dram_tensor("ws1", [C, 2 * C], F32, kind="ExternalInput")
    ws2 = nc.dram_tensor("ws2", [C, 2 * C], F32, kind="ExternalInput")
    idn = nc.dram_tensor("idn", [128, 128], F32, kind="ExternalInput")
    outD = nc.dram_tensor("out", [C, NPIX], F32, kind="ExternalOutput")

    with TileContext(nc) as tc:
        with (
            tc.tile_pool(name="const", bufs=1) as cp,
            tc.tile_pool(name="work", bufs=1) as wp,
            tc.tile_pool(name="tmp", bufs=1) as tp,
            tc.tile_pool(name="gat", bufs=2) as gp,
            tc.tile_pool(name="io", bufs=2) as iop,
            tc.tile_pool(name="ps", bufs=2, space="PSUM") as psp,
            tc.tile_pool(name="ps1", bufs=1, space="PSUM") as ps1p,
        ):
            # ---- constants ----
            wo_sb = cp.tile([128, K2 * 27], F32, tag="wo")
            nc.sync.dma_start(out=wo_sb, in_=wo[:])
            bo_sb = cp.tile([27, 1], F32, tag="bo")
            nc.sync.dma_start(out=bo_sb, in_=bo[:])
            wd_sb = cp.tile([128, 5 * C], F32, tag="wd")
            nc.sync.dma_start(out=wd_sb, in_=wd[:])
            ws1_sb = cp.tile([C, 2 * C], F32, tag="ws1")
            nc.sync.dma_start(out=ws1_sb, in_=ws1[:])
            ws2_sb = cp.tile([C, 2 * C], F32, tag="ws2")
            nc.sync.dma_start(out=ws2_sb, in_=ws2[:])
            idn_sb = cp.tile([128, 128], F32, tag="idn")
            nc.sync.dma_start(out=idn_sb, in_=idn[:])
            cy_sb = cp.tile([128, NTK], F32, tag="cy")
            nc.sync.dma_start(out=cy_sb, in_=cyf[:])
            cx_sb = cp.tile([128, NTK], F32, tag="cx")
            nc.sync.dma_start(out=cx_sb, in_=cxf[:])

            # ---- conv input: cat(x, inter) channel-major, padded columns ----
            ci = wp.tile([128, NROW * RW], F32, tag="ci")
            nc.vector.memset(fap(ci, 0, [[RW, NROW], [RW - 1, 2]]), 0.0)
            nc.sync.dma_start(
                out=fap(ci, 1, [[RW, NROW], [1, W]], p0=0, pn=C),
                in_=xi[:].rearrange("c (r w) -> c r w", w=W),
            )
            nc.sync.dma_start(
                out=fap(ci, 1, [[RW, NROW], [1, W]], p0=C, pn=C),
                in_=ii[:].rearrange("c (r w) -> c r w", w=W),
            )

            # ---- A: offset conv + transpose to pixel-major om_pm ----
            om_pm = gp.tile([128, HHALF * 27], F32, tag="samp", bufs=2)
            for nt in range(16):  # 4-row (512-pix) conv tiles
                r0 = nt * 4
                pconv = psp.tile([27, 512], F32, tag="dcnps")
                for tap in range(K2):
                    ky, kx = tap // 3, tap % 3
                    nc.tensor.matmul(
                        out=pconv,
                        lhsT=wo_sb[:, tap * 27:(tap + 1) * 27],
                        rhs=fap(ci, (r0 + ky) * RW + kx, [[RW, 4], [1, W]]),
                        start=(tap == 0),
                        stop=(tap == K2 - 1),
                    )
                oms = iop.tile([27, 512], F32, tag="oms")
                nc.scalar.activation(out=oms, in_=pconv, func=_AF.Identity,
                                     bias=bo_sb, scale=1.0)
                for q in range(4):
                    t = r0 + q
                    pt = ps1p.tile([128, 27], F32, tag="pa2")
                    nc.tensor.transpose(out=pt, in_=oms[:, q * 128:(q + 1) * 128],
                                        identity=idn_sb[0:27, 0:27])
                    nc.vector.tensor_copy(out=om_pm[:, t * 27:(t + 1) * 27],
                                          in_=pt)

            if phase == 1:
                nc.sync.dma_start(out=outD[:, 0:HHALF * 27], in_=om_pm[0:C, :])

            if phase >= 2:
                # ---- B: weight & index pipeline ----
                def dyx(off):  # [128, (tile, tap)] strided view of om_pm
                    return fap(om_pm, off, [[27, HHALF], [1, K2]])

                _tmp_n = [0]

                def tmp(tag, dt=F32):
                    _tmp_n[0] += 1
                    return tp.tile([128, NTK], dt, tag=tag,
                                   name=f"tmp_{tag}_{_tmp_n[0]}")

                MSK = tmp("ca")
                nc.scalar.activation(out=MSK, in_=dyx(18), func=_AF.Sigmoid)
                if phase == 2:
                    DBG1 = wp.tile([128, NTK], F32, tag="dbg1")
                    DBG2 = wp.tile([128, NTK], F32, tag="dbg2")
                    DBG3 = wp.tile([128, NTK], F32, tag="dbg3")
                    nc.vector.tensor_copy(out=DBG1, in_=MSK)
                PY = tmp("cb")
                nc.vector.tensor_add(out=PY, in0=dyx(0), in1=cy_sb)

                def floor_(src, scr1, scr2, dst):
                    ti = tmp("ci32", I32)
                    nc.vector.tensor_copy(out=ti, in_=src)
                    cf = tmp(scr1)
                    nc.vector.tensor_copy(out=cf, in_=ti)
                    gt = tmp(scr2)
                    nc.vector.tensor_tensor(out=gt, in0=cf, in1=src, op=_AL.is_gt)
                    fl = tmp(dst)
                    nc.vector.tensor_sub(out=fl, in0=cf, in1=gt)
                    return fl

                Y0 = floor_(PY, "cc", "cd", "ce")
                FY = tmp("cd")
                nc.vector.tensor_sub(out=FY, in0=PY, in1=Y0)
                if phase == 2:
                    nc.vector.tensor_copy(out=DBG2, in_=FY)
                PX = tmp("cb")
                nc.vector.tensor_add(out=PX, in0=dyx(9), in1=cx_sb)
                X0 = floor_(PX, "cc", "cf", "cg")
                FX = tmp("cf")
                nc.vector.tensor_sub(out=FX, in0=PX, in1=X0)
                if phase == 2:
                    nc.vector.tensor_copy(out=DBG3, in_=FX)

                GY1 = tmp("cb")
                nc.vector.tensor_mul(out=GY1, in0=FY, in1=MSK)
                GY0 = tmp("cc")
                nc.vector.tensor_sub(out=GY0, in0=MSK, in1=GY1)

                def rngmul(dst, src, lo, hi, s1, s2):
                    a = tmp(s1)
                    nc.vector.tensor_scalar(out=a, in0=src, scalar1=float(lo),
                                            scalar2=None, op0=_AL.is_ge)
                    b = tmp(s2)
                    nc.vector.tensor_scalar(out=b, in0=src, scalar1=float(hi),
                                            scalar2=None, op0=_AL.is_le)
                    nc.vector.tensor_mul(out=a, in0=a, in1=b)
                    nc.vector.tensor_mul(out=dst, in0=dst, in1=a)

                rngmul(GY0, Y0, 0.0, H - 1, "ca", "ch")   # MSK (ca) dead now
                rngmul(GY1, Y0, -1.0, H - 2, "ca", "ch")

                FX0 = tmp("ca")
                nc.vector.tensor_scalar(out=FX0, in0=FX, scalar1=-1.0, scalar2=1.0,
                                        op0=_AL.mult, op1=_AL.add)
                rngmul(FX0, X0, 0.0, W - 1, "ch", "cd")
                FX1 = tmp("cd")
                nc.vector.tensor_copy(out=FX1, in_=FX)
                rngmul(FX1, X0, -1.0, W - 2, "ch", "cf")      # FX (cf) dead now

                WA = wp.tile([128, NTK], F32, tag="wa")
                nc.vector.tensor_mul(out=WA, in0=GY0, in1=FX0)
                WB = wp.tile([128, NTK], F32, tag="wb")
                nc.vector.tensor_mul(out=WB, in0=GY0, in1=FX1)
                WC = wp.tile([128, NTK], F32, tag="wc")
                nc.vector.tensor_mul(out=WC, in0=GY1, in1=FX0)
                WD = wp.tile([128, NTK], F32, tag="wd4")
                nc.vector.tensor_mul(out=WD, in0=GY1, in1=FX1)

                YB0 = tmp("ca")
                nc.vector.tensor_scalar(out=YB0, in0=Y0, scalar1=0.0,
                                        scalar2=float(H - 1), op0=_AL.max, op1=_AL.min)
                YB1 = tmp("cb")  # clip(Y0,-1,H-2); +1 folded into G1f's add-term
                nc.vector.tensor_scalar(out=YB1, in0=Y0, scalar1=-1.0,
                                        scalar2=float(H - 2), op0=_AL.max, op1=_AL.min)
                XB = tmp("cc")
                nc.vector.tensor_scalar(out=XB, in0=X0, scalar1=-1.0,
                                        scalar2=float(W - 1), op0=_AL.max, op1=_AL.min)

                # g01 free layout: t*18 + row*9 + tap (matches idx value order)
                g01 = wp.tile([128, 2 * NTK], I16, tag="g01")
                for row, YB in ((0, YB0), (1, YB1)):
                    gf = tmp("ce" if row == 0 else "cg")
                    nc.vector.tensor_scalar(out=gf, in0=YB, scalar1=float(W),
                                            scalar2=float(W * (1 + row)),
                                            op0=_AL.mult, op1=_AL.add)
                    nc.vector.tensor_add(out=gf, in0=gf, in1=XB)
                    nc.vector.tensor_copy(
                        out=fap(g01, row * K2, [[2 * K2, HHALF], [1, K2]]),
                        in_=gf[:].rearrange("p (t k) -> p t k", k=K2))

                # ---- fold idx to wrapped-16 layout + replicate across Q7 cores ----
                idx = wp.tile([128, HHALF * 2 * K2 * 8], I16, tag="idx")
                for phi in range(8):
                    nc.sync.dma_start(
                        out=fap(idx, phi, [[8, 2 * NTK]], pn=16),
                        in_=fap(g01, 0, [[1, 2 * NTK]], p0=16 * phi, pn=16),
                    )
                for r in range(1, 8):
                    nc.sync.dma_start(out=idx[16 * r:16 * (r + 1)], in_=idx[0:16])

                if phase == 2:
                    nc.sync.dma_start(out=outD[:, 0:NTK], in_=WB[0:C, :])
                    nc.sync.dma_start(out=outD[:, NTK:2 * NTK], in_=WA[0:C, :])
                    nc.sync.dma_start(out=outD[:, 2 * NTK:3 * NTK],
                                      in_=WC[0:C, :])
                    nc.sync.dma_start(out=outD[:, 3 * NTK:4 * NTK],
                                      in_=WD[0:C, :])
                    nc.sync.dma_start(out=outD[:, 4 * NTK:5 * NTK],
                                      in_=DBG1[0:C, :])
                    nc.sync.dma_start(out=outD[:, 5 * NTK:6 * NTK],
                                      in_=DBG2[0:C, :])
                    nc.sync.dma_start(out=outD[:, 6 * NTK:7 * NTK],
                                      in_=DBG3[0:C, :])

                xg_ap = AP(xg, 0, [[C, XG_ROWS - 1], [1, 2 * C]])


            # ---- chunk loop ----
            for ch in range(N_CHUNKS if phase >= 3 else 0):
                t0 = ch * T_CHUNK
                g = gp.tile([128, NGRP * 128], F32, tag="g")
                nc.gpsimd.dma_gather(
                    out_ap=g[:].rearrange("p (n e) -> p n e", e=128),
                    in_ap=xg_ap,
                    idxs_ap=idx[:, ch * T_CHUNK * 144:(ch + 1) * T_CHUNK * 144],
                    num_idxs=NIDX,
                    num_idxs_reg=NIDX,
                    elem_size=2 * C,
                    elem_step=C,
                    single_packet=False,
                )

                if phase == 3:
                    nc.sync.dma_start(out=outD[:, 0:NGRP * 128], in_=g[0:C, :])
                    break

                def corner(row, half):
                    return fap(g, row * K2 * 128 + half * C,
                               [[2 * K2 * 128, T_CHUNK], [128, K2], [1, C]])

                def wbc(wt):
                    return fap(wt, t0 * K2,
                               [[K2, T_CHUNK], [1, K2], [0, C]])

                samp = gp.tile([128, T_CHUNK * 10 * C], F32, tag="samp")
                nc.vector.memset(fap(samp, 9 * C, [[10 * C, T_CHUNK], [1, C]]),
                                 0.0)
                sampv = fap(samp, 0, [[10 * C, T_CHUNK], [C, K2], [1, C]])
                t2 = gp.tile([128, T_CHUNK * K2 * C], F32, tag="t2", bufs=1)
                t2v = t2[:].rearrange("p (t k c) -> p t k c", k=K2, c=C)
                nc.vector.tensor_mul(out=sampv, in0=corner(0, 0), in1=wbc(WA))
                nc.vector.tensor_mul(out=t2v, in0=corner(0, 1), in1=wbc(WB))
                nc.vector.tensor_add(out=sampv, in0=sampv, in1=t2v)
                nc.vector.tensor_mul(out=t2v, in0=corner(1, 0), in1=wbc(WC))
                nc.vector.tensor_add(out=sampv, in0=sampv, in1=t2v)
                nc.vector.tensor_mul(out=t2v, in0=corner(1, 1), in1=wbc(WD))
                nc.vector.tensor_add(out=sampv, in0=sampv, in1=t2v)

                sampT = gp.tile([128, 5 * NPC], F32, tag="sampT", bufs=1)
                for t in range(T_CHUNK):
                    pa = ps1p.tile([128, 512], F32, tag="pa1", bufs=2)
                    for s in range(4):
                        nc.tensor.transpose(
                            out=pa[:, s * 128:(s + 1) * 128],
                            in_=samp[:, t * 10 * C + s * 128:
                                     t * 10 * C + (s + 1) * 128],
                            identity=idn_sb,
                        )
                    nc.vector.tensor_copy(
                        out=fap(sampT, t * 128, [[NPC, 4], [1, 128]]),
                        in_=pa,
                    )
                    pb = ps1p.tile([128, 128], F32, tag="pa2")
                    nc.tensor.transpose(
                        out=pb,
                        in_=samp[:, t * 10 * C + 512:t * 10 * C + 640],
                        identity=idn_sb,
                    )
                    nc.vector.tensor_copy(
                        out=sampT[:, 4 * NPC + t * 128:4 * NPC + (t + 1) * 128],
                        in_=pb,
                    )

                if phase == 4:
                    nc.sync.dma_start(out=outD[:, 0:5 * NPC], in_=sampT[0:C, :])
                    break

                # DCN + beta2 accumulate
                pdcn = psp.tile([C, NPC], F32, tag="dcnps")
                for s in range(5):
                    nc.tensor.matmul(
                        out=pdcn,
                        lhsT=wd_sb[:, s * C:(s + 1) * C],
                        rhs=sampT[:, s * NPC:(s + 1) * NPC],
                        start=(s == 0),
                        stop=False,
                    )

                it = iop.tile([C, NPC], F32, tag="it")
                nc.sync.dma_start(
                    out=it, in_=AP(ii, (t0 + 1) * W, [[NROW * W, C], [1, NPC]]))

                pg1 = ps1p.tile([C, NPC], F32, tag="s1")
                nc.tensor.matmul(out=pg1, lhsT=ws1_sb[:, 0:C], rhs=it,
                                 start=True, stop=True)
                u1s = iop.tile([C, NPC], F32, tag="u1s")
                nc.vector.tensor_scalar(out=u1s, in0=pg1, scalar1=0.1,
                                        scalar2=None, op0=_AL.mult)
                u1g = iop.tile([C, NPC], F32, tag="u1g")
                nc.vector.tensor_tensor(out=u1g, in0=pg1, in1=u1s, op=_AL.max)

                pb1 = ps1p.tile([C, NPC], F32, tag="s1")
                nc.tensor.matmul(out=pb1, lhsT=ws1_sb[:, C:2 * C], rhs=it,
                                 start=True, stop=True)
                u1t = iop.tile([C, NPC], F32, tag="u1s")
                nc.vector.tensor_scalar(out=u1t, in0=pb1, scalar1=0.1,
                                        scalar2=None, op0=_AL.mult)
                u1b = iop.tile([C, NPC], F32, tag="u1b")
                nc.vector.tensor_tensor(out=u1b, in0=pb1, in1=u1t, op=_AL.max)

                pg2 = ps1p.tile([C, NPC], F32, tag="s2")
                nc.tensor.matmul(out=pg2, lhsT=ws2_sb[:, 0:C], rhs=u1g,
                                 start=True, stop=True)
                nc.tensor.matmul(out=pdcn, lhsT=ws2_sb[:, C:2 * C], rhs=u1b,
                                 start=False, stop=True)

                xt = fap(ci, (t0 + 1) * RW + 1, [[RW, T_CHUNK], [1, W]], pn=C)
                ot = iop.tile([C, NPC], F32, tag="ot")
                nc.vector.tensor_mul(out=ot, in0=xt, in1=pg2)
                nc.vector.tensor_add(out=ot, in0=ot, in1=xt)
                nc.vector.tensor_add(out=ot, in0=ot, in1=pdcn)
                nc.sync.dma_start(out=outD[:, ch * NPC:(ch + 1) * NPC], in_=ot)

    nc.finalize()
    return nc


def _host_inputs(x, inter, w_offset, b_offset, w_dcn, w_g1, w_g2, w_b1, w_b2):
    x = np.ascontiguousarray(x, dtype=np.float32)
    inter = np.ascontiguousarray(inter, dtype=np.float32)

    # offset conv: [27, 2C, 3, 3] -> [2C, (tap, 27)]
    wo = np.transpose(w_offset, (2, 3, 1, 0))  # [ky, kx, 2C, 27]
    wo = wo.reshape(K2, 2 * C, 27).transpose(1, 0, 2).reshape(2 * C, K2 * 27)
    wo = np.ascontiguousarray(wo, dtype=np.float32)
    bo = np.ascontiguousarray(np.asarray(b_offset).reshape(27, 1), np.float32)

    wdk = np.asarray(w_dcn, np.float32).reshape(C, C, K2)  # [oc, c, tap]
    wd = np.zeros((2 * C, 5 * C), np.float32)
    for s in range(5):
        for j in range(2):
            tap = 2 * s + j
            if tap < K2:
                wd[j * C:(j + 1) * C, s * C:(s + 1) * C] = wdk[:, :, tap].T
    ws1 = np.concatenate([np.asarray(w_g1, np.float32).T,
                          np.asarray(w_b1, np.float32).T], axis=1)
    ws2 = np.concatenate([np.asarray(w_g2, np.float32).T,
                          np.asarray(w_b2, np.float32).T], axis=1)
    idn = np.eye(128, dtype=np.float32)

    pcol = np.arange(128, dtype=np.float32)
    tt = np.arange(HHALF, dtype=np.float32)
    kyv = np.arange(K2, dtype=np.float32) // 3
    kxv = np.arange(K2, dtype=np.float32) % 3
    cx = (pcol[:, None, None] + kxv[None, None, :] - 1.0
          + 0.0 * tt[None, :, None]).reshape(128, NTK).astype(np.float32)
    cx = np.ascontiguousarray(cx)

    in_maps = []
    for core in range(NCORES):
        b = core // 2
        h0 = (core % 2) * HHALF

        xgb = np.zeros((XG_ROWS, C), np.float32)
        xgb[W:W + H * W] = x[b].transpose(1, 2, 0).reshape(H * W, C)

        def halo(src):
            out = np.zeros((C, NROW, W), np.float32)
            lo, hi = h0 - 1, h0 + HHALF + 1
            slo, shi = max(lo, 0), min(hi, H)
            out[:, slo - lo:(slo - lo) + (shi - slo), :] = src[b][:, slo:shi, :]
            return np.ascontiguousarray(out.reshape(C, NROW * W))

        cy = (h0 + tt[None, :, None] + kyv[None, None, :] - 1.0
              + 0.0 * pcol[:, None, None]).reshape(128, NTK)
        cy = np.ascontiguousarray(cy.astype(np.float32))

        in_maps.append({
            "xg": xgb, "xi": halo(x), "ii": halo(inter),
            "cyf": cy, "cxf": cx,
            "wo": wo, "bo": bo, "wd": wd,
            "ws1": ws1, "ws2": ws2, "idn": idn,
        })
    return in_maps


_NC_CACHE = {}


def _get_nc():
    if "nc" not in _NC_CACHE:
        _NC_CACHE["nc"] = build_nc()
    return _NC_CACHE["nc"]


def kernel(x, inter, w_offset, b_offset, w_dcn, w_g1, w_g2, w_b1, w_b2,
           _trace=False):
    from concourse.bass_utils import run_bass_kernel_spmd

    in_maps = _host_inputs(x, inter, w_offset, b_offset, w_dcn,
                           w_g1, w_g2, w_b1, w_b2)
    nc = _get_nc()
    res = run_bass_kernel_spmd(nc, in_maps, list(range(NCORES)), trace=_trace)
    out = np.empty((B, C, H, W), np.float32)
    for core in range(NCORES):
        b = core // 2
        h0 = (core % 2) * HHALF
        out[b, :, h0:h0 + HHALF, :] = res.results[core]["out"].reshape(
            C, HHALF, W)
    if _trace:
        return out, res
    return out

